# revision 10
# baseline (speedup 1.0000x reference)
"""Multi-region RNN kernel for Trainium2 (8 NeuronCores, SPMD batch-sharded).

Model (per step t):
    inp  = einsum('bi,rih->rbh', x_t, W_ih)
    loc  = einsum('rbh,rhg->rbg', H, W_hh)
    msg  = einsum('ij,ibh->jbh', C, H)
    cross= einsum('rbh,rhg->rbg', msg, W_rhh)
    H'   = tanh(inp + loc + cross + bias)
Output: stack H over t -> [T,B,R*H] @ W_out + b_out.

Distribution: pure data-parallel over batch (B=32 -> 4 per core), parameters
replicated; no cross-core communication and no per-step DRAM traffic.

Per-core design (v2, fully SBUF-resident):
  State layout is region-major-columns [h=128, (r=100, b=4)] (col = r*4+b).
  One big SBUF tile `hist` [h, (t, r, b)] holds the input drive for every
  step (phase 1 writes it), and each step's tanh OVERWRITES slice t with
  H(t) in place -- it then serves as the state history phase 3 reads.

  Phase 1: per region one matmul (W_ih[r] stationary, x^T moving, all
  T*BL=512 cols) -> PSUM, then one bias-folding copy PSUM->hist[.,:,r,:]
  (ScalarE activation w/ bias port for odd r, DVE tensor_scalar_add for
  even r).

  Phase 2 (recurrence), per step, everything near the PE:
    - input drive + bias injected into PSUM via an identity matmul
      (start=True), split in two half-banks lo/hi;
    - H(t-1) -> region-major Hrm via 4 PE-transposes (identity) into a
      bf16 PSUM tile + 4 DVE copies to SBUF (no DMA transposes at all);
    - loc: 100 per-region matmuls accumulating into pa;
    - msg: 4 matmuls (lhsT = Hrm_b stationary, rhs = C), ScalarE copy
      PSUM->SBUF bf16;
    - cross: 100 per-region matmuls accumulating into pa;
    - tanh in two halves (lo can start while cross-hi still runs),
      writing H(t) into hist[t] in place.

  Phase 3: per 32-step block, 100 matmuls (W_out[r] stationary, hist
  moving) accumulating out^T [o, (t,b)] in PSUM; output stored transposed
  [O, T, BL]; host transposes back and adds b_out.
"""

import numpy as np
import ml_dtypes
from contextlib import ExitStack

import concourse.bass as bass
import concourse.bacc as bacc
import concourse.tile as tile
from concourse import mybir
from concourse import masks
from concourse.bass_utils import run_bass_kernel_spmd

T, B, I, H, R, O = 128, 32, 128, 128, 100, 64
NCORES = 8
BL = B // NCORES          # batch per core = 4
SC = R * BL               # state cols = 400, col = r*BL + b
TB = T * BL               # 512
TBLK = 32                 # t-steps per phase-3 block
HSC = SC // 2             # half of the state columns (r < 50 | r >= 50)

BF = mybir.dt.bfloat16
F32 = mybir.dt.float32
Act = mybir.ActivationFunctionType

_CACHE: dict = {}


def _build_program():
    nc = bacc.Bacc(None, target_bir_lowering=False)

    xT_d = nc.dram_tensor("xT", [I, TB], BF, kind="ExternalInput")        # [i,(t,b)]
    C_d = nc.dram_tensor("C", [R, R], BF, kind="ExternalInput")           # [i,j]
    Whh_d = nc.dram_tensor("Whh", [H, R * H], BF, kind="ExternalInput")   # [h,(r,g)]
    Wrhh_d = nc.dram_tensor("Wrhh", [H, R * H], BF, kind="ExternalInput")
    Wih_d = nc.dram_tensor("Wih", [I, R * H], BF, kind="ExternalInput")
    Wout_d = nc.dram_tensor("Wout", [H, R * O], BF, kind="ExternalInput")  # [h,(r,o)]
    biasT_d = nc.dram_tensor("biasT", [H, R], F32, kind="ExternalInput")
    out_d = nc.dram_tensor("out", [O, T, BL], F32, kind="ExternalOutput")  # transposed

    with tile.TileContext(nc) as tc, ExitStack() as ctx:
        consts = ctx.enter_context(tc.tile_pool(name="consts", bufs=1))

        # phase-1 critical consts first so their DMAs land early
        Wih_s = consts.tile([I, R * H], BF)
        nc.sync.dma_start(Wih_s[:], Wih_d[:])
        xT_s = consts.tile([I, TB], BF)
        nc.sync.dma_start(xT_s[:], xT_d[:])
        biasT_s = consts.tile([H, R], F32)
        nc.sync.dma_start(biasT_s[:], biasT_d[:])
        C_s = consts.tile([R, R], BF)
        nc.sync.dma_start(C_s[:], C_d[:])
        Whh_s = consts.tile([H, R * H], BF)
        nc.scalar.dma_start(Whh_s[:], Whh_d[:])
        Wrhh_s = consts.tile([H, R * H], BF)
        nc.scalar.dma_start(Wrhh_s[:], Wrhh_d[:])
        Wout_s = consts.tile([H, R * O], BF)
        nc.scalar.dma_start(Wout_s[:], Wout_d[:])

        ident_s = consts.tile([H, H], BF)
        masks.make_identity(nc, ident_s[:])

        # warm the ScalarE tanh activation table during phase 1 so the first
        # recurrence step doesn't pay the ~1.3us ACT_TABLE_LOAD
        warm_s = consts.tile([H, 1], BF)
        nc.scalar.activation(out=warm_s[:], in_=ident_s[:, 0:1], func=Act.Tanh)

        # input drive for all t; overwritten in place by H(t) at step t
        hist = consts.tile([H, T * SC], BF)
        hist4 = hist.rearrange("h (t r b) -> h t r b", r=R, b=BL)

        # ---------------- Phase 1: input drive + bias ----------------
        with tc.tile_pool(name="p1ps", bufs=2, space="PSUM") as p1ps:
            for r in range(R):
                ps = p1ps.tile([H, TB], F32, tag="p1")
                ps3 = ps.rearrange("h (t b) -> h t b", b=BL)
                nc.tensor.matmul(
                    ps[:], Wih_s[:, r * H:(r + 1) * H], xT_s[:],
                    start=True, stop=True,
                )
                if r % 2 == 0:
                    nc.vector.tensor_scalar_add(
                        hist4[:, :, r, :], ps3[:], biasT_s[:, r:r + 1])
                else:
                    nc.scalar.activation(
                        out=hist4[:, :, r, :], in_=ps3[:],
                        func=Act.Identity, bias=biasT_s[:, r:r + 1], scale=1.0,
                    )

        # ---------------- Phase 2: recurrence ----------------
        ps_pa = ctx.enter_context(tc.tile_pool(name="ps_pa", bufs=2, space="PSUM"))
        ps_pm = ctx.enter_context(tc.tile_pool(name="ps_pm", bufs=2, space="PSUM"))
        ps_tr = ctx.enter_context(tc.tile_pool(name="ps_tr", bufs=1, space="PSUM"))
        hrm_pool = ctx.enter_context(tc.tile_pool(name="hrm", bufs=2))
        msg_pool = ctx.enter_context(tc.tile_pool(name="msg", bufs=2))

        # The Tile scheduler's cost model treats LDWEIGHTS as free, so its
        # greedy list schedule front-loads the cheap-looking loc matmuls and
        # pushes the transpose->Hrm->msg->Msg chain far too late, exposing
        # its latency as a per-step PE stall. tile_set_cur_wait release
        # floors take manual control: each step gets a 20us sim window with
        # staged releases so the per-engine queue order is exactly the
        # software pipeline below.
        STEP_US = 20.0

        def W(t, us):
            tc.tile_set_cur_wait((100.0 + STEP_US * t + us) / 1000.0)

        for t in range(T):
            pa_lo = ps_pa.tile([H, 512], F32, tag="palo")
            pa_hi = ps_pa.tile([H, 512], F32, tag="pahi")
            inp_t = hist[:, t * SC:(t + 1) * SC]
            # inject input drive (+bias, already folded) into PSUM
            W(t, 0.0)
            nc.tensor.matmul(pa_lo[:, 0:HSC], ident_s[:], inp_t[:, 0:HSC],
                             start=True, stop=(t == 0))
            nc.tensor.matmul(pa_hi[:, 0:HSC], ident_s[:], inp_t[:, HSC:SC],
                             start=True, stop=(t == 0))
            if t > 0:
                Hp = hist[:, (t - 1) * SC:t * SC]
                Hp3 = Hp.rearrange("h (r b) -> h r b", b=BL)
                paL = pa_lo.rearrange("h (r b) -> h r b", b=BL)   # r in [0,128)
                paH = pa_hi.rearrange("h (r b) -> h r b", b=BL)

                def loc(r):
                    pdst = paL[:, r, :] if r < R // 2 else paH[:, r - R // 2, :]
                    nc.tensor.matmul(
                        pdst, Whh_s[:, r * H:(r + 1) * H], Hp3[:, r, :],
                        start=False, stop=False,
                    )

                # loc head covers the tanh-hi(t-1) latency before transposes
                for r in range(0, 12):
                    loc(r)

                # H(t-1) -> region-major via PE transposes + one fat DVE copy
                ptr = ps_tr.tile([H, BL * H], BF, tag="ptr")
                Hrm = hrm_pool.tile([H, BL * H], BF, tag="hrm")
                pm = ps_pm.tile([H, 512], F32, tag="pm")
                pmR = pm.rearrange("h (j b) -> h j b", b=BL)
                Msg = msg_pool.tile([H, SC], BF, tag="msg")
                W(t, 0.2)
                for b in range(BL):
                    nc.tensor.matmul(
                        ptr[0:R, b * H:(b + 1) * H], Hp3[:, :, b], ident_s[:],
                        is_transpose=True, start=(b == 0), stop=(b == BL - 1),
                    )
                W(t, 0.4)
                nc.vector.tensor_copy(Hrm[0:R, :], ptr[0:R, :])

                W(t, 1.6)
                for r in range(12, 37):
                    loc(r)

                # msg: pm[h, (j, b)] = sum_i H(t-1)[i, b, h] * C[i, j]
                W(t, 3.0)
                for b in range(BL):
                    nc.tensor.matmul(
                        pmR[:, 0:R, b], Hrm[0:R, b * H:(b + 1) * H], C_s[:],
                        start=(b == 0), stop=(b == BL - 1),
                    )
                # PSUM->SBUF msg eviction split across ScalarE / DVE so the
                # cross-lo wait is only on the ScalarE half
                W(t, 3.4)
                nc.scalar.activation(out=Msg[:, 0:HSC], in_=pm[:, 0:HSC],
                                     func=Act.Copy, scale=1.0)
                nc.vector.tensor_copy(Msg[:, HSC:SC], pm[:, HSC:SC])
                MsgR = Msg.rearrange("h (r b) -> h r b", b=BL)

                W(t, 4.0)
                for r in range(37, R):
                    loc(r)

                W(t, 6.5)
                for r in range(R // 2):
                    nc.tensor.matmul(
                        paL[:, r, :], Wrhh_s[:, r * H:(r + 1) * H], MsgR[:, r, :],
                        start=False, stop=(r == R // 2 - 1),
                    )
                W(t, 8.5)
                for r in range(R // 2, R):
                    nc.tensor.matmul(
                        paH[:, r - R // 2, :], Wrhh_s[:, r * H:(r + 1) * H],
                        MsgR[:, r, :], start=False, stop=(r == R - 1),
                    )

            # tanh halves -> H(t) written in place over the input drive
            W(t, 10.5)
            nc.scalar.activation(out=inp_t[:, 0:HSC], in_=pa_lo[:, 0:HSC],
                                 func=Act.Tanh)
            W(t, 11.0)
            nc.scalar.activation(out=inp_t[:, HSC:SC], in_=pa_hi[:, 0:HSC],
                                 func=Act.Tanh)

        # ---------------- Phase 3: output projection ----------------
        tc.tile_set_cur_wait((100.0 + STEP_US * T) / 1000.0)
        p3_ps = ctx.enter_context(tc.tile_pool(name="p3ps", bufs=1, space="PSUM"))
        p3_ot = ctx.enter_context(tc.tile_pool(name="p3ot", bufs=2))
        NBLK = T // TBLK
        for g in range(NBLK):
            po = p3_ps.tile([H, TBLK * BL], F32, tag="po")
            for r in range(R):
                nc.tensor.matmul(
                    po[0:O, :], Wout_s[:, r * O:(r + 1) * O],
                    hist4[:, g * TBLK:(g + 1) * TBLK, r, :],
                    start=(r == 0), stop=(r == R - 1),
                )
            ot = p3_ot.tile([O, TBLK * BL], F32, tag="ot")
            nc.scalar.activation(out=ot[:], in_=po[0:O, :], func=Act.Copy, scale=1.0)
            nc.sync.dma_start(
                out=out_d[:, g * TBLK:(g + 1) * TBLK, :],
                in_=ot.rearrange("o (t b) -> o t b", b=BL),
            )

    nc.compile()
    return nc


def _prep_inputs(x, C, W_ih, W_hh, W_rhh, bias, W_out):
    bf = ml_dtypes.bfloat16
    shared = {
        "C": np.ascontiguousarray(C).astype(bf),
        "Whh": np.ascontiguousarray(W_hh.transpose(1, 0, 2).reshape(H, R * H)).astype(bf),
        "Wrhh": np.ascontiguousarray(W_rhh.transpose(1, 0, 2).reshape(H, R * H)).astype(bf),
        "Wih": np.ascontiguousarray(W_ih.transpose(1, 0, 2).reshape(I, R * H)).astype(bf),
        "Wout": np.ascontiguousarray(
            W_out.reshape(R, H, O).transpose(1, 0, 2).reshape(H, R * O)
        ).astype(bf),
        "biasT": np.ascontiguousarray(bias.T).astype(np.float32),
    }
    in_maps = []
    for c in range(NCORES):
        xc = x[:, c * BL:(c + 1) * BL, :]                     # [T, BL, I]
        xT = np.ascontiguousarray(xc.transpose(2, 0, 1).reshape(I, TB)).astype(bf)
        m = dict(shared)
        m["xT"] = xT
        in_maps.append(m)
    return in_maps


def kernel(x, C, W_ih, W_hh, W_rhh, bias, W_out, b_out, _trace=False):
    x = np.asarray(x, np.float32)
    b_out = np.asarray(b_out, np.float32)
    in_maps = _prep_inputs(
        x, np.asarray(C, np.float32), np.asarray(W_ih, np.float32),
        np.asarray(W_hh, np.float32), np.asarray(W_rhh, np.float32),
        np.asarray(bias, np.float32), np.asarray(W_out, np.float32),
    )
    if "nc" not in _CACHE:
        _CACHE["nc"] = _build_program()
    nc = _CACHE["nc"]
    res = run_bass_kernel_spmd(nc, in_maps, list(range(NCORES)), trace=_trace)
    out = np.empty((T, B, O), np.float32)
    for c in range(NCORES):
        out[:, c * BL:(c + 1) * BL, :] = (
            res.results[c]["out"].transpose(1, 2, 0) + b_out
        )
    if _trace:
        return out, res
    return out
